# revision 6
# baseline (speedup 1.0000x reference)
"""Self-contained Trainium2 Bass kernel for the 3-layer GCN
(nn_Decoder_64020782514981): kernel(**inputs) -> np.ndarray.

Accepts FULL inputs, shards nodes across the 8 NeuronCores internally
(graph/data parallel), runs a Bass/Tile kernel via run_bass_kernel_spmd,
and returns the FULL [20000, 128] float32 output.

Per layer (A = adjacency + self loops, dinv = deg^-1/2):
    h_out = relu( dinv * (A^T (dinv * (h W))) + b )

Sharding: nodes are split into 8 contiguous ranges (2500 per core). Each
core transforms its own rows (z = dinv*(h@W)), the z shards are
all-gathered into a per-core DRAM table, and each core aggregates the
messages for its own destination rows by:
  - dma_gather of the source rows for its (dst-sorted, 128-padded) edges
  - a one-hot selection matmul per 128-edge chunk accumulating in PSUM.

The one-hot selection matrices are identical across layers: they are
built once on the vector engine into a persistent SBUF tile and reused.
Transform/aggregation matmuls run in bf16 (weights and features are
converted host-side); epilogues (dinv scaling + relu) run on the scalar
(Activation) engine, keeping DVE free for the transpose copies.

Host-side prep is pure index plumbing: edge bucketing by (core, dst
block), padding to chunk multiples, degree counting, and layout packing.
All FLOPs over features run on device.
"""
import numpy as np
import ml_dtypes

from concourse import bass, bacc, mybir
import concourse.tile as tile

P = 128

F32 = mybir.dt.float32
BF16 = mybir.dt.bfloat16
F32R = mybir.dt.float32r

BF16NP = ml_dtypes.bfloat16


class Cfg:
    def __init__(self, N, E, HID, OUT, n_cores, cpb, has_bias,
                 table_dt="bf16", mm_dt="bf16", grp=2, hoist_sel=True):
        self.skip_cc = False
        self.ablate = set()
        self.N, self.E, self.HID, self.OUT = N, E, HID, OUT
        self.NC = n_cores
        self.SH = N // n_cores             # nodes per core
        self.NT = (self.SH + P - 1) // P   # node tiles (= dst blocks) per core
        self.KC = HID // P                 # feature chunks of 128
        self.CPB = cpb                     # edge chunks per dst block (padded)
        self.has_bias = has_bias
        self.table_dt = table_dt
        self.mm_dt = mm_dt
        self.GRP = grp
        self.hoist_sel = hoist_sel


def prep(x, edge_index, W1, b1, W2, b2, W3, b3, n_cores=8,
         table_dt="bf16", mm_dt="bf16", grp=2, hoist_sel=True):
    """Shard inputs across cores; returns (cfg, in_maps)."""
    N, HID = x.shape
    OUT = W3.shape[1]
    E = edge_index.shape[1]
    SH = N // n_cores
    NT = (SH + P - 1) // P

    src = np.asarray(edge_index[0], dtype=np.int64)
    dst = np.asarray(edge_index[1], dtype=np.int64)

    deg = np.bincount(dst, minlength=N).astype(np.float32) + 1.0  # + self loop
    dinv = (1.0 / np.sqrt(deg)).astype(np.float32)

    has_bias = bool(np.any(b1) or np.any(b2) or np.any(b3))

    # Bucket edges by (core, dst block); append self-loop edges per block.
    # Order within a block is irrelevant (the selection matmul handles it).
    order = np.argsort(dst, kind="stable")
    src_s, dst_s = src[order], dst[order]

    buckets = []  # (core, block) -> (src_ids, dst_local)
    for c in range(n_cores):
        lo = c * SH
        for b in range(NT):
            blk_lo = lo + b * P
            blk_hi = min(lo + (b + 1) * P, lo + SH)
            i0 = np.searchsorted(dst_s, blk_lo)
            i1 = np.searchsorted(dst_s, blk_hi)
            bsrc = src_s[i0:i1]
            bdl = (dst_s[i0:i1] - blk_lo).astype(np.int64)
            # self loops
            loops = np.arange(blk_lo, blk_hi, dtype=np.int64)
            bsrc = np.concatenate([bsrc, loops])
            bdl = np.concatenate([bdl, loops - blk_lo])
            buckets.append((bsrc, bdl))

    cpb = max((len(b[0]) + P - 1) // P for b in buckets)

    cfg = Cfg(N, E, HID, OUT, n_cores, cpb, has_bias, table_dt, mm_dt,
              grp, hoist_sel)

    mmnp = BF16NP if mm_dt == "bf16" else np.float32
    iota = np.tile(np.arange(P, dtype=np.float32), (P, 1))
    ident = np.eye(P, dtype=mmnp)
    B1 = np.tile(np.asarray(b1, np.float32), (P, 1))
    B2 = np.tile(np.asarray(b2, np.float32), (P, 1))
    B3 = np.tile(np.asarray(b3, np.float32), (P, 1))

    in_maps = []
    for c in range(n_cores):
        lo = c * SH
        # gather indices, wrapped: idxs[p, s] = I[s*16 + (p % 16)]
        idxs_core = np.zeros((P, NT * cpb * 8), dtype=np.int16)
        dstloc_core = np.full((P, NT * cpb), -1.0, dtype=np.float32)
        for b in range(NT):
            bsrc, bdl = buckets[c * NT + b]
            n = len(bsrc)
            npad = cpb * P
            I = np.zeros(npad, dtype=np.int16)
            I[:n] = bsrc.astype(np.int16)
            D = np.full(npad, -1.0, dtype=np.float32)
            D[:n] = bdl.astype(np.float32)
            w16 = I.reshape(cpb * 8, 16).T  # [16, cpb*8]
            idxs_core[:, b * cpb * 8:(b + 1) * cpb * 8] = np.tile(w16, (8, 1))
            dstloc_core[:, b * cpb:(b + 1) * cpb] = D.reshape(cpb, P).T

        # dinv packed [P, NT]: node lo + t*128 + p -> [p, t]
        dinvT = np.ones((P, NT), dtype=np.float32)
        dv = dinv[lo:lo + SH]
        pad = NT * P - SH
        dvp = np.concatenate([dv, np.ones(pad, np.float32)])
        dinvT[:, :] = dvp.reshape(NT, P).T

        xT = np.ascontiguousarray(x[lo:lo + SH].T).astype(mmnp)  # [HID, SH]

        in_maps.append({
            "xT": xT,
            "idxs": idxs_core,
            "dstloc": dstloc_core,
            "dinvT": dinvT,
            "W1": np.asarray(W1).astype(mmnp), "W2": np.asarray(W2).astype(mmnp),
            "W3": np.asarray(W3).astype(mmnp),
            "B1": B1, "B2": B2, "B3": B3,
            "iota": iota, "ident": ident,
        })
    return cfg, in_maps


def build(cfg: Cfg) -> bass.Bass:
    N, HID, OUT = cfg.N, cfg.HID, cfg.OUT
    SH, NT, KC, CPB = cfg.SH, cfg.NT, cfg.KC, cfg.CPB

    tdt = {"f32": F32, "f32r": F32R, "bf16": BF16}[cfg.table_dt]
    mmdt = {"f32": F32, "f32r": F32R, "bf16": BF16}[cfg.mm_dt]

    nc = bacc.Bacc(None, target_bir_lowering=False, num_devices=cfg.NC,
                   num_swdge_queues=4)

    # I/O
    xT_in = nc.declare_dram_parameter("xT", [HID, SH], mmdt, isOutput=False)
    idxs_in = nc.declare_dram_parameter("idxs", [P, NT * CPB * 8], mybir.dt.int16, isOutput=False)
    dstloc_in = nc.declare_dram_parameter("dstloc", [P, NT * CPB], F32, isOutput=False)
    dinvT_in = nc.declare_dram_parameter("dinvT", [P, NT], F32, isOutput=False)
    W_in = [nc.declare_dram_parameter(f"W{i+1}", [HID, HID if i < 2 else OUT], mmdt, isOutput=False) for i in range(3)]
    B_in = [nc.declare_dram_parameter(f"B{i+1}", [P, HID if i < 2 else OUT], F32, isOutput=False) for i in range(3)]
    iota_in = nc.declare_dram_parameter("iota", [P, P], F32, isOutput=False)
    ident_in = nc.declare_dram_parameter("ident", [P, P], mmdt, isOutput=False)
    out_ext = nc.declare_dram_parameter("out", [SH, OUT], F32, isOutput=True)

    # Internal DRAM
    z_local = [nc.dram_tensor(f"z{L}_local", [SH, HID if L < 2 else OUT], tdt) for L in range(3)]
    z_table = [nc.dram_tensor(f"z{L}_table", [N, HID if L < 2 else OUT], tdt, addr_space="Shared") for L in range(3)]

    core_ids = list(range(cfg.NC))

    # All cores must enter this execution before touching shared collective
    # state — prevents cross-iteration desync when the NEFF is executed
    # back-to-back (profiling replays). Emitted outside the TileContext so
    # the tile scheduling sim doesn't see an unsatisfiable wait.
    if not cfg.skip_cc:
        nc.gpsimd.bir_kernel_barrier_wait([core_ids])

    GRP = cfg.GRP

    with tile.TileContext(nc) as tc:
        with (
            tc.tile_pool(name="persist", bufs=1) as pp,
            tc.tile_pool(name="msg", bufs=3) as msg_pool,
            tc.tile_pool(name="sel", bufs=6) as sel_pool,
            tc.tile_pool(name="zsb", bufs=3) as zsb_pool,
            tc.tile_pool(name="hsb", bufs=3) as hsb_pool,
            tc.tile_pool(name="psz", bufs=2, space="PSUM") as psz_pool,
            tc.tile_pool(name="psa", bufs=4, space="PSUM") as psa_pool,
            tc.tile_pool(name="pst", bufs=2, space="PSUM") as pst_pool,
        ):
            # ---- resident tiles ----
            hT_a = pp.tile([P, KC, SH], mmdt, tag="hta")
            hT_b = pp.tile([P, KC, SH], mmdt, tag="htb")
            W_sb = [pp.tile([P, KC, HID if i < 2 else OUT], mmdt, tag=f"w{i}", name=f"W_sb{i}") for i in range(3)]
            B_sb = [pp.tile([P, HID if i < 2 else OUT], F32, tag=f"b{i}", name=f"B_sb{i}") for i in range(3)]
            idxs_sb = pp.tile([P, NT * CPB * 8], mybir.dt.int16, tag="idxs")
            dstloc_sb = pp.tile([P, NT * CPB], F32, tag="dstloc")
            dinv_sb = pp.tile([P, NT], F32, tag="dinv")
            iota_sb = pp.tile([P, P], F32, tag="iota")
            ident_sb = pp.tile([P, P], mmdt, tag="ident")
            if cfg.hoist_sel:
                S_all = pp.tile([P, NT * CPB, P], tdt, tag="sall")

            nidx_regs = {gb: nc.gpsimd.to_reg(gb * CPB * P)
                         for gb in sorted({min(GRP, NT - g) for g in range(0, NT, GRP)})}

            nc.sync.dma_start(out=hT_a[:], in_=xT_in[:].rearrange("(c p) n -> p c n", p=P))
            for i in range(3):
                nc.sync.dma_start(out=W_sb[i][:], in_=W_in[i][:].rearrange("(c p) o -> p c o", p=P))
            if cfg.has_bias:
                for i in range(3):
                    nc.sync.dma_start(out=B_sb[i][:], in_=B_in[i][:])
            nc.sync.dma_start(out=idxs_sb[:], in_=idxs_in[:])
            nc.sync.dma_start(out=dstloc_sb[:], in_=dstloc_in[:])
            nc.sync.dma_start(out=dinv_sb[:], in_=dinvT_in[:])
            nc.sync.dma_start(out=iota_sb[:], in_=iota_in[:])
            nc.sync.dma_start(out=ident_sb[:], in_=ident_in[:])

            if cfg.hoist_sel and "nosel" not in cfg.ablate:
                # One-hot selection matrices, shared across all 3 layers.
                # Built per dst block so the scheduler can interleave them
                # with layer-0 transform matmuls.
                for b in range(NT):
                    nc.vector.tensor_tensor(
                        out=S_all[:, b * CPB:(b + 1) * CPB, :],
                        in0=dstloc_sb[:, b * CPB:(b + 1) * CPB][:, :, None]
                            .to_broadcast([P, CPB, P]),
                        in1=iota_sb[:][:, None, :].to_broadcast([P, CPB, P]),
                        op=mybir.AluOpType.is_equal,
                    )

            for L in range(3):
                OW = HID if L < 2 else OUT
                hT_cur = hT_a if L % 2 == 0 else hT_b
                hT_next = hT_b if L % 2 == 0 else hT_a

                # ---- transform: z = dinv * (h @ W_L) ----
                for t in range(NT):
                    npt = min(P, SH - t * P)
                    sl = slice(t * P, t * P + npt)
                    psz = psz_pool.tile([P, OW], F32, tag="psz")
                    for kc in range(KC):
                        nc.tensor.matmul(
                            out=psz[:npt, :],
                            lhsT=hT_cur[:, kc, sl],
                            rhs=W_sb[L][:, kc, :],
                            start=(kc == 0), stop=(kc == KC - 1),
                        )
                    z_sb = zsb_pool.tile([P, OW], tdt, tag="zsb")
                    nc.scalar.activation(
                        out=z_sb[:npt, :], in_=psz[:npt, :],
                        func=mybir.ActivationFunctionType.Copy,
                        scale=dinv_sb[:npt, t:t + 1],
                    )
                    nc.sync.dma_start(out=z_local[L][sl, :], in_=z_sb[:npt, :])

                # ---- all-gather z shards (Tile tracks the DRAM deps) ----
                if cfg.skip_cc:
                    # timing-only mode: skip the collective (WRONG results)
                    nc.sync.dma_start(out=z_table[L][:SH, :], in_=z_local[L][:])
                else:
                    nc.gpsimd.collective_compute(
                        "AllGather", mybir.AluOpType.bypass,
                        ins=[z_local[L][:].opt()], outs=[z_table[L][:].opt()],
                        replica_groups=[core_ids],
                    )

                # ---- aggregation, one gather per group of dst blocks ----
                for g in range(0, NT, GRP):
                    gb = min(GRP, NT - g)
                    msg = msg_pool.tile([P, GRP * CPB, OW], tdt, tag="msg")
                    if "nogather" not in cfg.ablate:
                        nc.gpsimd.dma_gather(
                            out_ap=msg[:, :gb * CPB, :],
                            in_ap=z_table[L][:],
                            idxs_ap=idxs_sb[:, g * CPB * 8:(g + gb) * CPB * 8],
                            num_idxs=gb * CPB * P,
                            num_idxs_reg=nidx_regs[gb],
                            elem_size=OW,
                            single_packet=False,
                            queue_num=(L * NT + g) % 4,
                        )
                    for b in range(g, g + gb):
                        npt = min(P, SH - b * P)
                        sl = slice(b * P, b * P + npt)
                        mo = (b - g) * CPB
                        agg = psa_pool.tile([P, OW], F32, tag="agg")
                        if cfg.hoist_sel:
                            S = S_all[:, b * CPB:(b + 1) * CPB, :]
                        else:
                            St = sel_pool.tile([P, CPB, P], tdt, tag="sel")
                            nc.vector.tensor_tensor(
                                out=St[:],
                                in0=dstloc_sb[:, b * CPB:(b + 1) * CPB][:, :, None]
                                    .to_broadcast([P, CPB, P]),
                                in1=iota_sb[:][:, None, :].to_broadcast([P, CPB, P]),
                                op=mybir.AluOpType.is_equal,
                            )
                            S = St[:]
                        nkc = 1 if "noagg" in cfg.ablate else CPB
                        for k in range(nkc):
                            nc.tensor.matmul(
                                out=agg[:],
                                lhsT=S[:, k, :],
                                rhs=msg[:, mo + k, :],
                                start=(k == 0), stop=(k == nkc - 1),
                            )
                        # ---- epilogue ----
                        if L < 2:
                            h_sb = hsb_pool.tile([P, OW], mmdt, tag="hsb")
                            nc.scalar.activation(
                                out=h_sb[:npt, :], in_=agg[:npt, :],
                                func=mybir.ActivationFunctionType.Relu,
                                scale=dinv_sb[:npt, b:b + 1])
                            if cfg.has_bias:
                                raise NotImplementedError("bias unsupported")
                            # transpose into hT_next
                            for fc in range(KC):
                                pst = pst_pool.tile([P, P], F32, tag="pst")
                                nc.tensor.transpose(
                                    out=pst[:, :npt],
                                    in_=h_sb[:npt, fc * P:(fc + 1) * P],
                                    identity=ident_sb[:npt, :npt])
                                nc.vector.tensor_copy(
                                    out=hT_next[:, fc, sl], in_=pst[:, :npt])
                        else:
                            h_sb = hsb_pool.tile([P, OW], F32, tag="hsb")
                            nc.scalar.activation(
                                out=h_sb[:npt, :], in_=agg[:npt, :],
                                func=mybir.ActivationFunctionType.Copy,
                                scale=dinv_sb[:npt, b:b + 1])
                            nc.sync.dma_start(out=out_ext[sl, :], in_=h_sb[:npt, :])

    nc.finalize()
    split_sync_waits(nc)
    return nc


_MAXW = 1
_counter = [0]


def split_sync_waits(nc, maxw=_MAXW):
    n_split = 0
    for f in nc.m.functions:
        for bb in f.blocks:
            insts = list(bb.instructions)
            out = []
            changed = False
            for inst in insts:
                si = inst.sync_info
                eff = maxw
                if si is not None and len(si.on_wait) > eff:
                    waits = list(si.on_wait)
                    keep = waits[-eff:] if eff else []
                    rest = waits[: len(waits) - eff]
                    for w in rest:
                        _counter[0] += 1
                        nop = mybir.InstNoOp(
                            name=f"wspill-{_counter[0]}",
                            engine=inst.engine,
                            bass_nofuse=True,
                            sync_info=mybir.SyncInfo(on_wait=[w], on_update=[]),
                        )
                        nc.register_instruction(nop)
                        out.append(nop)
                    si.on_wait = keep
                    changed = True
                    n_split += 1
                out.append(inst)
            if changed:
                bb.instructions = out
    return n_split


def kernel(**inputs):
    from concourse.bass_utils import run_bass_kernel_spmd

    x = np.asarray(inputs["x"], dtype=np.float32)
    edge_index = np.asarray(inputs["edge_index"])
    cfg, in_maps = prep(
        x, edge_index,
        np.asarray(inputs["W1"], np.float32), np.asarray(inputs["b1"], np.float32),
        np.asarray(inputs["W2"], np.float32), np.asarray(inputs["b2"], np.float32),
        np.asarray(inputs["W3"], np.float32), np.asarray(inputs["b3"], np.float32),
        n_cores=8, table_dt="bf16", mm_dt="bf16")
    nc = build(cfg)
    res = run_bass_kernel_spmd(nc, in_maps, core_ids=list(range(cfg.NC)))
    out = np.concatenate([res.results[c]["out"] for c in range(cfg.NC)], axis=0)
    return out.astype(np.float32)


# revision 7
# speedup vs baseline: 2.0470x; 2.0470x over previous
"""Self-contained Trainium2 Bass kernel for the 3-layer GCN
(nn_Decoder_64020782514981): kernel(**inputs) -> np.ndarray.

Accepts FULL inputs, shards nodes across the 8 NeuronCores internally
(graph/data parallel), runs a Bass/Tile kernel via run_bass_kernel_spmd,
and returns the FULL [20000, 128] float32 output.

Per layer (A = adjacency + self loops, dinv = deg^-1/2):
    h_out = relu( dinv * (A^T (dinv * (h W))) + b )

Sharding: nodes are split into 8 contiguous ranges (2500 per core). Each
core transforms its own rows (z = dinv*(h@W)), the z shards are
all-gathered into a per-core DRAM table, and each core aggregates the
messages for its own destination rows by:
  - dma_gather of the source rows for its (dst-sorted, 128-padded) edges
  - a one-hot selection matmul per 128-edge chunk accumulating in PSUM.

The one-hot selection matrices are identical across layers: they are
built once on the vector engine into a persistent SBUF tile and reused.
Transform/aggregation matmuls run in bf16 (weights and features are
converted host-side); epilogues (dinv scaling + relu) run on the scalar
(Activation) engine, keeping DVE free for the transpose copies.

Host-side prep is pure index plumbing: edge bucketing by (core, dst
block), padding to chunk multiples, degree counting, and layout packing.
All FLOPs over features run on device.
"""
import numpy as np
import ml_dtypes

from concourse import bass, bacc, mybir
import concourse.tile as tile

P = 128

F32 = mybir.dt.float32
BF16 = mybir.dt.bfloat16
F32R = mybir.dt.float32r

BF16NP = ml_dtypes.bfloat16


class Cfg:
    def __init__(self, N, E, HID, OUT, n_cores, cpb, has_bias,
                 table_dt="bf16", mm_dt="bf16", grp=2, hoist_sel=True):
        self.skip_cc = False
        self.ablate = set()
        self.N, self.E, self.HID, self.OUT = N, E, HID, OUT
        self.NC = n_cores
        self.SH = N // n_cores             # nodes per core
        self.NT = (self.SH + P - 1) // P   # node tiles (= dst blocks) per core
        self.KC = HID // P                 # feature chunks of 128
        self.CPB = cpb                     # edge chunks per dst block (padded)
        self.has_bias = has_bias
        self.table_dt = table_dt
        self.mm_dt = mm_dt
        self.GRP = grp
        self.hoist_sel = hoist_sel


def prep(x, edge_index, W1, b1, W2, b2, W3, b3, n_cores=8,
         table_dt="bf16", mm_dt="bf16", grp=2, hoist_sel=True):
    """Shard inputs across cores; returns (cfg, in_maps)."""
    N, HID = x.shape
    OUT = W3.shape[1]
    E = edge_index.shape[1]
    SH = N // n_cores
    NT = (SH + P - 1) // P

    src = np.asarray(edge_index[0], dtype=np.int64)
    dst = np.asarray(edge_index[1], dtype=np.int64)

    deg = np.bincount(dst, minlength=N).astype(np.float32) + 1.0  # + self loop
    dinv = (1.0 / np.sqrt(deg)).astype(np.float32)

    has_bias = bool(np.any(b1) or np.any(b2) or np.any(b3))

    # Bucket edges by (core, dst block); append self-loop edges per block.
    # Order within a block is irrelevant (the selection matmul handles it).
    order = np.argsort(dst, kind="stable")
    src_s, dst_s = src[order], dst[order]

    buckets = []  # (core, block) -> (src_ids, dst_local)
    for c in range(n_cores):
        lo = c * SH
        for b in range(NT):
            blk_lo = lo + b * P
            blk_hi = min(lo + (b + 1) * P, lo + SH)
            i0 = np.searchsorted(dst_s, blk_lo)
            i1 = np.searchsorted(dst_s, blk_hi)
            bsrc = src_s[i0:i1]
            bdl = (dst_s[i0:i1] - blk_lo).astype(np.int64)
            # self loops
            loops = np.arange(blk_lo, blk_hi, dtype=np.int64)
            bsrc = np.concatenate([bsrc, loops])
            bdl = np.concatenate([bdl, loops - blk_lo])
            buckets.append((bsrc, bdl))

    cpb = max((len(b[0]) + P - 1) // P for b in buckets)

    cfg = Cfg(N, E, HID, OUT, n_cores, cpb, has_bias, table_dt, mm_dt,
              grp, hoist_sel)

    mmnp = BF16NP if mm_dt == "bf16" else np.float32
    iota = np.tile(np.arange(P, dtype=np.float32), (P, 1))
    ident = np.eye(P, dtype=mmnp)
    B1 = np.tile(np.asarray(b1, np.float32), (P, 1))
    B2 = np.tile(np.asarray(b2, np.float32), (P, 1))
    B3 = np.tile(np.asarray(b3, np.float32), (P, 1))

    in_maps = []
    for c in range(n_cores):
        lo = c * SH
        # gather indices, wrapped: idxs[p, s] = I[s*16 + (p % 16)]
        idxs_core = np.zeros((P, NT * cpb * 8), dtype=np.int16)
        dstloc_core = np.full((P, NT * cpb), -1.0, dtype=np.float32)
        for b in range(NT):
            bsrc, bdl = buckets[c * NT + b]
            n = len(bsrc)
            npad = cpb * P
            I = np.zeros(npad, dtype=np.int16)
            I[:n] = bsrc.astype(np.int16)
            D = np.full(npad, -1.0, dtype=np.float32)
            D[:n] = bdl.astype(np.float32)
            w16 = I.reshape(cpb * 8, 16).T  # [16, cpb*8]
            idxs_core[:, b * cpb * 8:(b + 1) * cpb * 8] = np.tile(w16, (8, 1))
            dstloc_core[:, b * cpb:(b + 1) * cpb] = D.reshape(cpb, P).T

        # dinv packed [P, NT]: node lo + t*128 + p -> [p, t]
        dinvT = np.ones((P, NT), dtype=np.float32)
        dv = dinv[lo:lo + SH]
        pad = NT * P - SH
        dvp = np.concatenate([dv, np.ones(pad, np.float32)])
        dinvT[:, :] = dvp.reshape(NT, P).T

        xT = np.ascontiguousarray(x[lo:lo + SH].T).astype(mmnp)  # [HID, SH]

        in_maps.append({
            "xT": xT,
            "idxs": idxs_core,
            "dstloc": dstloc_core,
            "dinvT": dinvT,
            "W1": np.asarray(W1).astype(mmnp), "W2": np.asarray(W2).astype(mmnp),
            "W3": np.asarray(W3).astype(mmnp),
            "B1": B1, "B2": B2, "B3": B3,
            "iota": iota, "ident": ident,
        })
    return cfg, in_maps


def build(cfg: Cfg) -> bass.Bass:
    N, HID, OUT = cfg.N, cfg.HID, cfg.OUT
    SH, NT, KC, CPB = cfg.SH, cfg.NT, cfg.KC, cfg.CPB

    tdt = {"f32": F32, "f32r": F32R, "bf16": BF16}[cfg.table_dt]
    mmdt = {"f32": F32, "f32r": F32R, "bf16": BF16}[cfg.mm_dt]

    nc = bacc.Bacc(None, target_bir_lowering=False, num_devices=cfg.NC,
                   num_swdge_queues=4)

    # I/O
    xT_in = nc.declare_dram_parameter("xT", [HID, SH], mmdt, isOutput=False)
    idxs_in = nc.declare_dram_parameter("idxs", [P, NT * CPB * 8], mybir.dt.int16, isOutput=False)
    dstloc_in = nc.declare_dram_parameter("dstloc", [P, NT * CPB], F32, isOutput=False)
    dinvT_in = nc.declare_dram_parameter("dinvT", [P, NT], F32, isOutput=False)
    W_in = [nc.declare_dram_parameter(f"W{i+1}", [HID, HID if i < 2 else OUT], mmdt, isOutput=False) for i in range(3)]
    B_in = [nc.declare_dram_parameter(f"B{i+1}", [P, HID if i < 2 else OUT], F32, isOutput=False) for i in range(3)]
    iota_in = nc.declare_dram_parameter("iota", [P, P], F32, isOutput=False)
    ident_in = nc.declare_dram_parameter("ident", [P, P], mmdt, isOutput=False)
    out_ext = nc.declare_dram_parameter("out", [SH, OUT], F32, isOutput=True)

    # Internal DRAM
    z_local = [nc.dram_tensor(f"z{L}_local", [SH, HID if L < 2 else OUT], tdt) for L in range(3)]
    z_table = [nc.dram_tensor(f"z{L}_table", [N, HID if L < 2 else OUT], tdt, addr_space="Shared") for L in range(3)]

    core_ids = list(range(cfg.NC))

    # All cores must enter this execution before touching shared collective
    # state — prevents cross-iteration desync when the NEFF is executed
    # back-to-back (profiling replays). Emitted outside the TileContext so
    # the tile scheduling sim doesn't see an unsatisfiable wait.
    if not cfg.skip_cc:
        nc.gpsimd.bir_kernel_barrier_wait([core_ids])

    GRP = cfg.GRP

    with tile.TileContext(nc) as tc:
        with (
            tc.tile_pool(name="persist", bufs=1) as pp,
            tc.tile_pool(name="msg", bufs=3) as msg_pool,
            tc.tile_pool(name="sel", bufs=6) as sel_pool,
            tc.tile_pool(name="zsb", bufs=3) as zsb_pool,
            tc.tile_pool(name="hsb", bufs=3) as hsb_pool,
            tc.tile_pool(name="psz", bufs=2, space="PSUM") as psz_pool,
            tc.tile_pool(name="psa", bufs=4, space="PSUM") as psa_pool,
            tc.tile_pool(name="pst", bufs=2, space="PSUM") as pst_pool,
        ):
            # ---- resident tiles ----
            hT_a = pp.tile([P, KC, SH], mmdt, tag="hta")
            hT_b = pp.tile([P, KC, SH], mmdt, tag="htb")
            W_sb = [pp.tile([P, KC, HID if i < 2 else OUT], mmdt, tag=f"w{i}", name=f"W_sb{i}") for i in range(3)]
            B_sb = [pp.tile([P, HID if i < 2 else OUT], F32, tag=f"b{i}", name=f"B_sb{i}") for i in range(3)]
            idxs_sb = pp.tile([P, NT * CPB * 8], mybir.dt.int16, tag="idxs")
            dstloc_sb = pp.tile([P, NT * CPB], F32, tag="dstloc")
            dinv_sb = pp.tile([P, NT], F32, tag="dinv")
            iota_sb = pp.tile([P, P], F32, tag="iota")
            ident_sb = pp.tile([P, P], mmdt, tag="ident")
            if cfg.hoist_sel:
                S_all = pp.tile([P, NT * CPB, P], tdt, tag="sall")

            nidx_regs = {gb: nc.gpsimd.to_reg(gb * CPB * P)
                         for gb in sorted({min(GRP, NT - g) for g in range(0, NT, GRP)})}

            nc.sync.dma_start(out=hT_a[:], in_=xT_in[:].rearrange("(c p) n -> p c n", p=P))
            for i in range(3):
                nc.sync.dma_start(out=W_sb[i][:], in_=W_in[i][:].rearrange("(c p) o -> p c o", p=P))
            if cfg.has_bias:
                for i in range(3):
                    nc.sync.dma_start(out=B_sb[i][:], in_=B_in[i][:])
            nc.sync.dma_start(out=idxs_sb[:], in_=idxs_in[:])
            nc.sync.dma_start(out=dstloc_sb[:], in_=dstloc_in[:])
            nc.sync.dma_start(out=dinv_sb[:], in_=dinvT_in[:])
            nc.sync.dma_start(out=iota_sb[:], in_=iota_in[:])
            nc.sync.dma_start(out=ident_sb[:], in_=ident_in[:])

            if cfg.hoist_sel and "nosel" not in cfg.ablate:
                # One-hot selection matrices, shared across all 3 layers.
                # Built per dst block so the scheduler can interleave them
                # with layer-0 transform matmuls.
                for b in range(NT):
                    nc.vector.tensor_tensor(
                        out=S_all[:, b * CPB:(b + 1) * CPB, :],
                        in0=dstloc_sb[:, b * CPB:(b + 1) * CPB][:, :, None]
                            .to_broadcast([P, CPB, P]),
                        in1=iota_sb[:][:, None, :].to_broadcast([P, CPB, P]),
                        op=mybir.AluOpType.is_equal,
                    )

            for L in range(3):
                OW = HID if L < 2 else OUT
                hT_cur = hT_a if L % 2 == 0 else hT_b
                hT_next = hT_b if L % 2 == 0 else hT_a

                # ---- transform: z = dinv * (h @ W_L) ----
                for t in range(NT):
                    npt = min(P, SH - t * P)
                    sl = slice(t * P, t * P + npt)
                    psz = psz_pool.tile([P, OW], F32, tag="psz")
                    for kc in range(KC):
                        nc.tensor.matmul(
                            out=psz[:npt, :],
                            lhsT=hT_cur[:, kc, sl],
                            rhs=W_sb[L][:, kc, :],
                            start=(kc == 0), stop=(kc == KC - 1),
                        )
                    z_sb = zsb_pool.tile([P, OW], tdt, tag="zsb")
                    nc.scalar.activation(
                        out=z_sb[:npt, :], in_=psz[:npt, :],
                        func=mybir.ActivationFunctionType.Copy,
                        scale=dinv_sb[:npt, t:t + 1],
                    )
                    nc.sync.dma_start(out=z_local[L][sl, :], in_=z_sb[:npt, :])

                # ---- all-gather z shards (Tile tracks the DRAM deps) ----
                if cfg.skip_cc:
                    # timing-only mode: skip the collective (WRONG results)
                    nc.sync.dma_start(out=z_table[L][:SH, :], in_=z_local[L][:])
                else:
                    nc.gpsimd.collective_compute(
                        "AllGather", mybir.AluOpType.bypass,
                        ins=[z_local[L][:].opt()], outs=[z_table[L][:].opt()],
                        replica_groups=[core_ids],
                    )

                # ---- aggregation, one gather per group of dst blocks ----
                for g in range(0, NT, GRP):
                    gb = min(GRP, NT - g)
                    msg = msg_pool.tile([P, GRP * CPB, OW], tdt, tag="msg")
                    if "nogather" not in cfg.ablate:
                        nc.gpsimd.dma_gather(
                            out_ap=msg[:, :gb * CPB, :],
                            in_ap=z_table[L][:],
                            idxs_ap=idxs_sb[:, g * CPB * 8:(g + gb) * CPB * 8],
                            num_idxs=gb * CPB * P,
                            num_idxs_reg=nidx_regs[gb],
                            elem_size=OW,
                            single_packet=False,
                            queue_num=(L * NT + g) % 4,
                        )
                    for b in range(g, g + gb):
                        npt = min(P, SH - b * P)
                        sl = slice(b * P, b * P + npt)
                        mo = (b - g) * CPB
                        agg = psa_pool.tile([P, OW], F32, tag="agg")
                        if cfg.hoist_sel:
                            S = S_all[:, b * CPB:(b + 1) * CPB, :]
                        else:
                            St = sel_pool.tile([P, CPB, P], tdt, tag="sel")
                            nc.vector.tensor_tensor(
                                out=St[:],
                                in0=dstloc_sb[:, b * CPB:(b + 1) * CPB][:, :, None]
                                    .to_broadcast([P, CPB, P]),
                                in1=iota_sb[:][:, None, :].to_broadcast([P, CPB, P]),
                                op=mybir.AluOpType.is_equal,
                            )
                            S = St[:]
                        nkc = 1 if "noagg" in cfg.ablate else CPB
                        for k in range(nkc):
                            nc.tensor.matmul(
                                out=agg[:],
                                lhsT=S[:, k, :],
                                rhs=msg[:, mo + k, :],
                                start=(k == 0), stop=(k == nkc - 1),
                            )
                        # ---- epilogue ----
                        if L < 2:
                            h_sb = hsb_pool.tile([P, OW], mmdt, tag="hsb")
                            nc.scalar.activation(
                                out=h_sb[:npt, :], in_=agg[:npt, :],
                                func=mybir.ActivationFunctionType.Relu,
                                scale=dinv_sb[:npt, b:b + 1])
                            if cfg.has_bias:
                                raise NotImplementedError("bias unsupported")
                            # transpose into hT_next
                            for fc in range(KC):
                                pst = pst_pool.tile([P, P], mmdt, tag="pst")
                                nc.tensor.transpose(
                                    out=pst[:, :npt],
                                    in_=h_sb[:npt, fc * P:(fc + 1) * P],
                                    identity=ident_sb[:npt, :npt])
                                nc.vector.tensor_copy(
                                    out=hT_next[:, fc, sl], in_=pst[:, :npt])
                        else:
                            h_sb = hsb_pool.tile([P, OW], F32, tag="hsb")
                            nc.scalar.activation(
                                out=h_sb[:npt, :], in_=agg[:npt, :],
                                func=mybir.ActivationFunctionType.Copy,
                                scale=dinv_sb[:npt, b:b + 1])
                            nc.sync.dma_start(out=out_ext[sl, :], in_=h_sb[:npt, :])

    nc.finalize()
    split_sync_waits(nc)
    return nc


_MAXW = 1
_counter = [0]


def split_sync_waits(nc, maxw=_MAXW):
    n_split = 0
    for f in nc.m.functions:
        for bb in f.blocks:
            insts = list(bb.instructions)
            out = []
            changed = False
            for inst in insts:
                si = inst.sync_info
                eff = maxw
                if si is not None and len(si.on_wait) > eff:
                    waits = list(si.on_wait)
                    keep = waits[-eff:] if eff else []
                    rest = waits[: len(waits) - eff]
                    for w in rest:
                        _counter[0] += 1
                        nop = mybir.InstNoOp(
                            name=f"wspill-{_counter[0]}",
                            engine=inst.engine,
                            bass_nofuse=True,
                            sync_info=mybir.SyncInfo(on_wait=[w], on_update=[]),
                        )
                        nc.register_instruction(nop)
                        out.append(nop)
                    si.on_wait = keep
                    changed = True
                    n_split += 1
                out.append(inst)
            if changed:
                bb.instructions = out
    return n_split


def kernel(**inputs):
    from concourse.bass_utils import run_bass_kernel_spmd

    x = np.asarray(inputs["x"], dtype=np.float32)
    edge_index = np.asarray(inputs["edge_index"])
    cfg, in_maps = prep(
        x, edge_index,
        np.asarray(inputs["W1"], np.float32), np.asarray(inputs["b1"], np.float32),
        np.asarray(inputs["W2"], np.float32), np.asarray(inputs["b2"], np.float32),
        np.asarray(inputs["W3"], np.float32), np.asarray(inputs["b3"], np.float32),
        n_cores=8, table_dt="bf16", mm_dt="bf16")
    nc = build(cfg)
    res = run_bass_kernel_spmd(nc, in_maps, core_ids=list(range(cfg.NC)))
    out = np.concatenate([res.results[c]["out"] for c in range(cfg.NC)], axis=0)
    return out.astype(np.float32)
